# revision 2
# baseline (speedup 1.0000x reference)
"""DeepSeekV2 single-token decode attention on 8 Trainium2 cores — v2.

Differences from the v1 baseline (all host prep is layout-only):
  - qkv trimmed to the 4224 features actually used (q 4096 + kv[:128]);
    TP slice 528/core.
  - Everything on the wire is bf16 (A2A payloads included).
  - All heavy DMAs are contiguous (host-swizzled layouts; vg pre-swizzled
    to the SBUF [p, c, e] layout; weights pre-swizzled to [p, hc, o]).
  - KV streamed in 32-chunk (1 MiB) pieces.
  - exp batched over groups of 8 chunks; masking via 0/1 multiply on the
    few masked chunks instead of per-chunk bias ACTs.
  - P4 in lhsT=weights orientation producing po^T [5120, 64]; the final
    reduce is AllToAll + on-chip block-sum; output stays feature-sharded
    [640, 64] per core and the host reassembles (replaces ReduceScatter).
"""

import ml_dtypes
import numpy as np

import concourse.bass as bass
import concourse.tile as tile
from concourse import bacc, mybir
from concourse.bass_utils import run_bass_kernel_spmd

F32 = mybir.dt.float32
BF16 = mybir.dt.bfloat16
EXP = mybir.ActivationFunctionType.Exp
BFNP = ml_dtypes.bfloat16

H, D = 32, 128
HID = 5120
Q_SIZE = H * D            # 4096
QKV_USED = Q_SIZE + D     # 4224 features actually used
B, S_MAX = 64, 8192
SCALE = float(D) ** -0.5
NC = 8
BPC = B // NC             # 8 seqs per core
OSL = QKV_USED // NC      # 528 qkv output slice per core
QSL = Q_SIZE // NC        # 512 w_o contraction rows per core
WSL = HID // NC           # 640 output cols per core (final shard)
HC = HID // 128           # 40 hidden chunks
NEG = -30000.0
PIECE = 32                # kv chunks per DMA piece (1 MiB)
GROUP = 8                 # score chunks per exp group

_BUILD_CACHE = {}
PROFILE = False
LAST_RES = None
REPEAT = 1


def _build(budgets, r0s):
    """budgets[j] = chunks for slot j; r0s[j] = first masked chunk."""
    nc = bacc.Bacc("TRN2", target_bir_lowering=False, debug=False,
                   num_devices=NC)

    hT = nc.dram_tensor("hT", [128, HC, B], BF16, kind="ExternalInput").ap()
    wq = nc.dram_tensor("wq", [128, HC, OSL], BF16, kind="ExternalInput").ap()
    wo = nc.dram_tensor("wo", [128, Q_SIZE // 128, WSL], BF16,
                        kind="ExternalInput").ap()
    ident = nc.dram_tensor("ident", [128, 128], BF16, kind="ExternalInput").ap()
    ntb = nc.dram_tensor("ntb", [1, BPC], F32, kind="ExternalInput").ap()
    kts, vgs, mgs = [], [], []
    for j in range(BPC):
        cj = budgets[j]
        kts.append(nc.dram_tensor(f"kt{j}", [128, cj * 128], BF16,
                                  kind="ExternalInput").ap())
        vgs.append(nc.dram_tensor(f"vg{j}", [128, cj, 129], BF16,
                                  kind="ExternalInput").ap())
        lj = cj - r0s[j]
        mgs.append(nc.dram_tensor(f"mg{j}", [128, lj], F32,
                                  kind="ExternalInput").ap())
    outp = nc.dram_tensor("outp", [WSL, B], F32, kind="ExternalOutput").ap()

    a1_in = nc.dram_tensor("a1_in", [QKV_USED, BPC], BF16).ap()
    a1_out = nc.dram_tensor("a1_out", [QKV_USED, BPC], BF16).ap()
    at_in = nc.dram_tensor("at_in", [Q_SIZE, BPC], BF16).ap()
    at_ag = nc.dram_tensor("at_ag", [NC * Q_SIZE, BPC], BF16).ap()

    groups = [list(range(NC))]
    OCS = [128, 128, 128, 128, OSL - 512]   # qkv slice partition chunks

    with tile.TileContext(nc) as tc:
        with (
            tc.tile_pool(name="wts", bufs=1) as wts_pool,
            tc.tile_pool(name="acts", bufs=1) as acts_pool,
            tc.tile_pool(name="kt", bufs=3) as kt_pool,
            tc.tile_pool(name="vg", bufs=3) as vg_pool,
            tc.tile_pool(name="probs", bufs=4) as probs_pool,
            tc.tile_pool(name="small", bufs=2) as small_pool,
            tc.tile_pool(name="psA", bufs=2, space="PSUM") as psA,
            tc.tile_pool(name="psSc", bufs=3, space="PSUM") as psSc,
            tc.tile_pool(name="psAt", bufs=1, space="PSUM") as psAt,
            tc.tile_pool(name="psX", bufs=1, space="PSUM") as psX,
        ):
          def body():
            # ---------- Prefetch stream (SP ring, consumption order) -------
            # All of these are dep-free: the SP sequencer issues them
            # back-to-back from t=0.  Dep-gated DMAs go on the ACT ring
            # (nc.scalar) so they never block this stream.
            hT_t = acts_pool.tile([128, HC, B], BF16, tag="hT")
            nc.sync.dma_start(hT_t[:, :, :], hT[:, :, :])
            id_t = acts_pool.tile([128, 128], BF16, tag="id")
            nc.sync.dma_start(id_t[:, :], ident[:, :])
            ntb_t = acts_pool.tile([1, BPC], F32, tag="ntb")
            nc.sync.dma_start(ntb_t[:, :], ntb[:, :])
            wq_t = wts_pool.tile([128, HC, OSL], BF16, tag="wq")
            nc.sync.dma_start(wq_t[:, :, :], wq[:, :, :])
            mg_ts = []
            for j in range(BPC):
                mg_t = small_pool.tile([128, budgets[j] - r0s[j]], F32,
                                       tag=f"mg{j}")
                nc.sync.dma_start(mg_t[:, :], mgs[j][:, :])
                mg_ts.append(mg_t)
            kv_tiles = []
            wo_t = None
            for j in range(BPC):
                cj = budgets[j]
                pieces = []
                for p0 in range(0, cj, PIECE):
                    pc = min(PIECE, cj - p0)
                    kt_t = kt_pool.tile([128, PIECE * 128], BF16, tag="kt")
                    nc.sync.dma_start(
                        kt_t[:, :pc * 128],
                        kts[j][:, p0 * 128:(p0 + pc) * 128])
                    vg_t = vg_pool.tile([128, PIECE, 129], BF16, tag="vg")
                    nc.sync.dma_start(vg_t[:, :pc, :], vgs[j][:, p0:p0 + pc, :])
                    pieces.append((p0, pc, kt_t, vg_t))
                kv_tiles.append(pieces)
                if j == 1:
                    wo_t = wts_pool.tile([128, Q_SIZE // 128, WSL], BF16,
                                         tag="wo")
                    nc.sync.dma_start(wo_t[:, :, :], wo[:, :, :])

            # ---------- Phase 1: qkv projection (TP column slice) ----------
            a1_view = a1_in.rearrange("(d o) j -> o d j", d=NC)  # [OSL, NC, BPC]
            for oc in range(5):
                osz = OCS[oc]
                ps_q = psA.tile([128, B], F32, tag="mm")
                for hc in range(HC):
                    nc.tensor.matmul(
                        ps_q[:osz, :],
                        lhsT=wq_t[:, hc, oc * 128:oc * 128 + osz],
                        rhs=hT_t[:, hc, :],
                        start=(hc == 0), stop=(hc == HC - 1),
                    )
                q_sb = small_pool.tile([128, B], BF16, tag="qsb")
                nc.vector.tensor_copy(q_sb[:osz, :], ps_q[:osz, :])
                nc.scalar.dma_start(
                    a1_view[oc * 128:oc * 128 + osz, :, :],
                    q_sb[:osz, :].rearrange("o (d j) -> o d j", d=NC),
                )

            nc.gpsimd.collective_compute(
                "AllToAll", mybir.AluOpType.bypass, replica_groups=groups,
                ins=[a1_in[:, :]], outs=[a1_out[:, :]],
            )

            # ---------- Phase 2: per-core q / kv_new ----------
            # qt[:, 0:32, j] = q^T per head; qt[:, 32, j] = kv_new^T
            qt_t = acts_pool.tile([128, H + 1, BPC], BF16, tag="qt")
            nc.scalar.dma_start(
                qt_t[:, :, :],
                a1_out[:, :].rearrange("(h p) j -> p h j", p=128),
            )
            # kv_new rows + ones column, via per-slot PE transpose (identity)
            kvnr_t = acts_pool.tile([1, BPC, 129], BF16, tag="kvnr")
            for j in range(BPC):
                ps_kv = psX.tile([1, 128], F32, tag="misc")
                nc.tensor.matmul(ps_kv[:, :], lhsT=qt_t[:, H, j:j + 1],
                                 rhs=id_t[:, :], start=True, stop=True)
                nc.vector.tensor_copy(kvnr_t[0:1, j, 0:128], ps_kv[:, :])
            nc.vector.memset(kvnr_t[:, :, 128:129], 1.0)

            at_all_t = acts_pool.tile([H, D, BPC], BF16, tag="atall")

            # ---------- Phase 3: attention, one slot at a time ----------
            for j in range(BPC):
                cj, r0 = budgets[j], r0s[j]
                mg_t = mg_ts[j]
                attn_ps = psAt.tile([H, D + 1], F32, tag="at")
                qt_j = qt_t[:, 0:H, j]
                n_mm = 0
                for (p0, pc, kt_t, vg_t) in kv_tiles[j]:
                    for g0 in range(0, pc, GROUP):
                        gs = min(GROUP, pc - g0)
                        ps_sc = psSc.tile([128, GROUP * H], F32, tag="sc")
                        for k in range(gs):
                            nc.tensor.matmul(
                                ps_sc[:, k * H:(k + 1) * H],
                                lhsT=kt_t[:, (g0 + k) * 128:(g0 + k + 1) * 128],
                                rhs=qt_j,
                                start=True, stop=True,
                            )
                        pt = probs_pool.tile([128, GROUP * H], BF16, tag="pt")
                        nc.scalar.activation(
                            pt[:, :gs * H], ps_sc[:, :gs * H], EXP, scale=SCALE)
                        for k in range(gs):
                            ch = p0 + g0 + k
                            if ch >= r0:
                                nc.vector.tensor_scalar_mul(
                                    pt[:, k * H:(k + 1) * H],
                                    pt[:, k * H:(k + 1) * H],
                                    mg_t[:, ch - r0:ch - r0 + 1],
                                )
                        for k in range(gs):
                            nc.tensor.matmul(
                                attn_ps[:, :],
                                lhsT=pt[:, k * H:(k + 1) * H],
                                rhs=vg_t[:, g0 + k, :],
                                start=(n_mm == 0), stop=False,
                            )
                            n_mm += 1
                # new-token term
                ps_nt = psX.tile([1, H], F32, tag="nt")
                nc.tensor.matmul(
                    ps_nt[:, :], lhsT=qt_t[:, H, j:j + 1], rhs=qt_j,
                    start=True, stop=True)
                pn_t = small_pool.tile([1, H], BF16, tag="pn")
                nc.scalar.activation(
                    pn_t[:, :], ps_nt[:, :], EXP,
                    bias=ntb_t[0:1, j:j + 1], scale=SCALE)
                nc.tensor.matmul(
                    attn_ps[:, :], lhsT=pn_t[0:1, :],
                    rhs=kvnr_t[0:1, j, :],
                    start=False, stop=True)

                rc_t = small_pool.tile([H, 1], F32, tag="rc")
                nc.vector.reciprocal(rc_t[:, :], attn_ps[:, D:D + 1])
                nc.vector.tensor_scalar_mul(
                    at_all_t[:, :, j], attn_ps[:, 0:D], rc_t[:, :])

            # ---------- Phase 4: AllGather attn + output projection --------
            nc.scalar.dma_start(
                at_in.rearrange("(h x) j -> h x j", h=H), at_all_t[:, :, :])
            nc.gpsimd.collective_compute(
                "AllGather", mybir.AluOpType.bypass, replica_groups=groups,
                ins=[at_in[:, :]], outs=[at_ag[:, :]],
            )

            QC = Q_SIZE // 128  # 32 contraction chunks
            ao_t = acts_pool.tile([128, QC, NC, BPC], BF16, tag="ao")
            ag_view = at_ag.rearrange("(c q p) j -> p q c j", p=128, c=NC)
            for c2 in range(NC):
                nc.scalar.dma_start(ao_t[:, :, c2, :], ag_view[:, :, c2, :])

            out_sb = acts_pool.tile([128, WSL // 128, B], F32, tag="osb")
            for w in range(WSL // 128):
                ps_o = psA.tile([128, B], F32, tag="mm")
                for qc in range(QC):
                    nc.tensor.matmul(
                        ps_o[:, :],
                        lhsT=wo_t[:, qc, w * 128:(w + 1) * 128],
                        rhs=ao_t[:, qc, :, :],
                        start=(qc == 0), stop=(qc == QC - 1),
                    )
                nc.vector.tensor_copy(out_sb[:, w, :], ps_o[:, :])
            nc.scalar.dma_start(
                outp.rearrange("(k p) b -> p k b", p=128), out_sb[:, :, :])

          for _rep in range(REPEAT):
              body()

    nc.compile()
    return nc


def _prepare(hidden_states, positions, kv_cache, slot_mapping, seq_lens,
             w_qkv, w_o):
    """Host-side sharding/layout prep. Returns (nc, in_maps, col_seq)."""
    hidden_states = np.asarray(hidden_states, dtype=np.float32)
    kv_cache = np.asarray(kv_cache, dtype=np.float32)
    sl = np.asarray(seq_lens).astype(np.int64)
    sm = np.asarray(slot_mapping).astype(np.int64)
    w_qkv = np.asarray(w_qkv, dtype=np.float32)
    w_o = np.asarray(w_o, dtype=np.float32)

    # sort by length desc, deal round-robin: core c slot j <- rank 8j + c
    order = np.argsort(-sl, kind="stable")
    seq_of = np.empty((NC, BPC), dtype=np.int64)
    for j in range(BPC):
        for c in range(NC):
            seq_of[c, j] = order[NC * j + c]
    col_seq = seq_of.reshape(NC * BPC)  # global column order (c, j)

    budgets, r0s = [], []
    for j in range(BPC):
        lens_j = sl[seq_of[:, j]]
        budgets.append(max(1, -(-int(lens_j.max()) // 128)))
        r0s.append(int((lens_j.min() - 1) // 128))
    budgets, r0s = tuple(budgets), tuple(r0s)

    key = (budgets, r0s, REPEAT)
    if key not in _BUILD_CACHE:
        _BUILD_CACHE[key] = _build(budgets, r0s)
    nc = _BUILD_CACHE[key]

    # shared (replicated) tensors
    hTs = hidden_states[col_seq, 0, :].T.reshape(HC, 128, B)
    hT_sw = np.ascontiguousarray(hTs.transpose(1, 0, 2)).astype(BFNP)
    ident = np.eye(128, dtype=BFNP)

    in_maps = []
    for c in range(NC):
        wq_c = w_qkv[c * OSL:(c + 1) * OSL, :]        # [528, HID]
        wq_sw = np.ascontiguousarray(
            wq_c.T.reshape(HC, 128, OSL).transpose(1, 0, 2)).astype(BFNP)
        wo_c = w_o[c * WSL:(c + 1) * WSL, :]          # [640 outs, 4096]
        wo_sw = np.ascontiguousarray(
            wo_c.T.reshape(Q_SIZE // 128, 128, WSL).transpose(1, 0, 2)
        ).astype(BFNP)
        m = {
            "hT": hT_sw,
            "wq": wq_sw,
            "wo": wo_sw,
            "ident": ident,
            "ntb": np.where(sm[seq_of[c]] < sl[seq_of[c]], 0.0, NEG
                            ).astype(np.float32).reshape(1, BPC),
        }
        for j in range(BPC):
            b = seq_of[c, j]
            L, slot = int(sl[b]), int(sm[b])
            n = budgets[j] * 128
            m[f"kt{j}"] = np.ascontiguousarray(
                kv_cache[0, b, :n, :].T).astype(BFNP)
            vg = np.empty((budgets[j], 128, 129), dtype=BFNP)
            vg[:, :, :128] = kv_cache[1, b, :n, :].reshape(budgets[j], 128, 128)
            vg[:, :, 128] = 1.0
            m[f"vg{j}"] = np.ascontiguousarray(vg.transpose(1, 0, 2))
            # 0/1 mask for chunks >= r0s[j]
            lj = budgets[j] - r0s[j]
            pos = (np.arange(r0s[j] * 128, n).reshape(lj, 128)).T  # [128, lj]
            mg = ((pos < L) & (pos != slot)).astype(np.float32)
            m[f"mg{j}"] = np.ascontiguousarray(mg)
        in_maps.append(m)

    return nc, in_maps, col_seq


def kernel(hidden_states, positions, kv_cache, slot_mapping, seq_lens,
           w_qkv, w_o):
    nc, in_maps, col_seq = _prepare(
        hidden_states, positions, kv_cache, slot_mapping, seq_lens,
        w_qkv, w_o)
    res = run_bass_kernel_spmd(nc, in_maps, list(range(NC)), trace=PROFILE)
    global LAST_RES
    LAST_RES = res

    out = np.empty((B, 1, HID), dtype=np.float32)
    for c in range(NC):
        shard = res.results[c]["outp"]        # [640, 64] = [my outs, (c2, j)]
        out[col_seq, 0, c * WSL:(c + 1) * WSL] = shard.T
    return out
